# revision 3
# baseline (speedup 1.0000x reference)
"""Trainium2 Bass kernel for KL-divergence 1-NN label lookup (AnchorStore).

reference:
    self[k]  = mean_d a[k,d]*log a[k,d]
    cross    = einsum('kd,bd->kb', a, log q) / D
    kl[b,k]  = self[k] - cross[k,b]
    out[b]   = queue_label[argmin_k kl[b,k]]

Strategy (8 NeuronCores, D-sharded):
    Each core owns a D-slice (padded with 1.0 so log()=0 contributes
    nothing). Per core, compute in SUM units (scale-invariant for argmin):
        m_partial[b,k] = sum_d lq[d,b]*at[d,k] - sum_d at[d,k]*log(at[d,k])
    via TensorE: stationary lq tiles [128d,128b] x moving at [128d,512k]
    accumulated in PSUM, plus a "-self" contribution from a (-1)-matrix
    stationary times t = at*log(at). AllReduce(add) the [256,2048] partials,
    then every core computes argmax_k m (== argmin kl) and the label via a
    max/is_equal mask against a broadcast label row. Output int32 labels.
"""

import os
import sys

import numpy as np

sys.path.insert(0, "/opt/trn_rl_repo")

from concourse import bacc, bass, mybir, tile  # noqa: E402
from concourse import bass_utils  # noqa: E402

K = 2048
B = 256
D = 50257
NCORES = 8
DSH = 6400  # padded per-core D-slice (50 tiles of 128)
F32 = mybir.dt.float32


def build(mm_dtype=F32, dsh=DSH, debug_out=False):
    """Build the SPMD Bass graph for one core (all cores identical)."""
    nt = dsh // 128  # d-tiles per core
    kc = K // 512    # k chunks (psum banks per b-tile)
    nc = bacc.Bacc(
        "TRN2", target_bir_lowering=False, debug=False, num_devices=NCORES
    )
    at_d = nc.dram_tensor("at", [dsh, K], F32, kind="ExternalInput")
    qt_d = nc.dram_tensor("qt", [dsh, B], F32, kind="ExternalInput")
    lab_d = nc.dram_tensor("lab1", [128, K], F32, kind="ExternalInput")
    out_d = nc.dram_tensor("out", [2, 128], mybir.dt.int32, kind="ExternalOutput")
    if debug_out:
        mdbg_d = nc.dram_tensor("mdbg", [B, K], F32, kind="ExternalOutput")

    def mm(ap):
        return ap if mm_dtype == F32 else ap.bitcast(mm_dtype)

    LN = mybir.ActivationFunctionType.Ln
    AX = mybir.AxisListType.X
    OP = mybir.AluOpType

    with tile.TileContext(nc) as tc:
        with (
            tc.tile_pool(name="const", bufs=1) as constp,
            tc.tile_pool(name="lqp", bufs=1) as lqp,
            tc.tile_pool(name="qinp", bufs=4) as qinp,
            tc.tile_pool(name="atp", bufs=4) as atp,
            tc.tile_pool(name="latp", bufs=2) as latp,
            tc.tile_pool(name="tpp", bufs=2) as tpp,
            tc.tile_pool(name="msbp", bufs=2) as msbp,
            tc.tile_pool(name="epp", bufs=2) as epp,
            tc.tile_pool(name="psp", bufs=1, space="PSUM") as psp,
            tc.tile_pool(name="dramp", bufs=1, space="DRAM") as dramp,
        ):
            negones = constp.tile([128, 128], F32)
            nc.gpsimd.memset(negones[:], -1.0)
            lab1 = constp.tile([128, K], F32)
            nc.sync.dma_start(lab1[:], lab_d[:])

            # lq = log(query^T), resident in SBUF: [128, nt*B]
            lq = lqp.tile([128, nt * B], F32)
            for t in range(nt):
                qtile = qinp.tile([128, B], F32)
                nc.sync.dma_start(qtile[:], qt_d[t * 128 : (t + 1) * 128, :])
                nc.scalar.activation(lq[:, t * B : (t + 1) * B], qtile[:], LN)

            # PSUM accumulators: [c][bt] -> [128b, 512k] (one bank each)
            pk = [
                [
                    psp.tile(
                        [128, 512],
                        F32,
                        name=f"pk_{c}_{bt}",
                        tag=f"pk_{c}_{bt}",
                    )
                    for bt in range(2)
                ]
                for c in range(kc)
            ]

            for t in range(nt):
                att = atp.tile([128, K], F32)
                nc.sync.dma_start(att[:], at_d[t * 128 : (t + 1) * 128, :])
                latt = latp.tile([128, K], F32)
                nc.scalar.activation(latt[:], att[:], LN)
                tt = tpp.tile([128, K], F32)
                nc.vector.tensor_tensor(tt[:], att[:], latt[:], op=OP.mult)
                for bt in range(2):
                    lhs = lq[:, t * B + bt * 128 : t * B + bt * 128 + 128]
                    for c in range(kc):
                        nc.tensor.matmul(
                            pk[c][bt][:],
                            mm(lhs),
                            mm(att[:, c * 512 : (c + 1) * 512]),
                            start=(t == 0),
                            stop=False,
                        )
                for bt in range(2):
                    for c in range(kc):
                        nc.tensor.matmul(
                            pk[c][bt][:],
                            mm(negones[:]),
                            mm(tt[:, c * 512 : (c + 1) * 512]),
                            start=False,
                            stop=(t == nt - 1),
                        )

            # PSUM -> SBUF partials
            m_sb = [
                msbp.tile([128, K], F32, name=f"m_sb{bt}", tag=f"m_sb{bt}")
                for bt in range(2)
            ]
            for bt in range(2):
                for c in range(kc):
                    nc.vector.tensor_copy(
                        m_sb[bt][:, c * 512 : (c + 1) * 512], pk[c][bt][:]
                    )

            # AllReduce(add) partial m across the 8 cores
            ar_in = dramp.tile([B, K], F32)
            ar_out = dramp.tile([B, K], F32, addr_space="Shared")
            for bt in range(2):
                nc.gpsimd.dma_start(
                    ar_in[bt * 128 : (bt + 1) * 128, :], m_sb[bt][:]
                )
            nc.gpsimd.collective_compute(
                "AllReduce",
                OP.add,
                replica_groups=[list(range(NCORES))],
                ins=[ar_in.opt()],
                outs=[ar_out.opt()],
            )

            # Epilogue: argmax over k and label extraction (identical on all
            # cores).
            for bt in range(2):
                msum = epp.tile([128, K], F32)
                nc.sync.dma_start(msum[:], ar_out[bt * 128 : (bt + 1) * 128, :])
                if debug_out:
                    nc.sync.dma_start(
                        mdbg_d[bt * 128 : (bt + 1) * 128, :], msum[:]
                    )
                gmax = epp.tile([128, 1], F32)
                nc.vector.tensor_reduce(gmax[:], msum[:], axis=AX, op=OP.max)
                eq = epp.tile([128, K], F32)
                nc.vector.tensor_scalar(
                    eq[:], msum[:], gmax[:], None, op0=OP.is_equal
                )
                cand = epp.tile([128, K], F32)
                nc.vector.tensor_tensor(cand[:], eq[:], lab1[:], op=OP.mult)
                lmax = epp.tile([128, 1], F32)
                nc.vector.tensor_reduce(lmax[:], cand[:], axis=AX, op=OP.max)
                labf = epp.tile([128, 1], F32)
                nc.vector.tensor_scalar_add(labf[:], lmax[:], -1.0)
                labi = epp.tile([128, 1], mybir.dt.int32)
                nc.vector.tensor_copy(labi[:], labf[:])
                nc.sync.dma_start(out_d[bt, :], labi[:])

    nc.compile()
    return nc


def shard_inputs(query, queue_anchor, queue_label, dsh=DSH, d_real=D):
    """Host-side layout prep: pad D with 1.0 (log 1 = 0), per-core
    transposed slices, broadcast label row."""
    q = np.asarray(query, np.float32)
    a = np.asarray(queue_anchor, np.float32)
    lab1 = (np.asarray(queue_label).astype(np.float32) + 1.0)[None, :]
    lab1 = np.ascontiguousarray(np.broadcast_to(lab1, (128, lab1.shape[1])))
    in_maps = []
    for c in range(NCORES):
        lo = c * dsh
        hi = min((c + 1) * dsh, d_real)
        at = np.ones((dsh, a.shape[0]), np.float32)
        qt = np.ones((dsh, q.shape[0]), np.float32)
        if hi > lo:
            at[: hi - lo, :] = a[:, lo:hi].T
            qt[: hi - lo, :] = q[:, lo:hi].T
        in_maps.append({"at": at, "qt": qt, "lab1": lab1})
    return in_maps


_NC_CACHE = {}


def _get_nc():
    key = os.environ.get("ANCHOR_MM_DTYPE", "float32r")
    if key not in _NC_CACHE:
        _NC_CACHE[key] = build(mm_dtype=getattr(mybir.dt, key))
    return _NC_CACHE[key]


def kernel(query, queue_anchor, queue_label):
    nc = _get_nc()
    in_maps = shard_inputs(query, queue_anchor, queue_label)
    res = bass_utils.run_bass_kernel_spmd(
        nc, in_maps, core_ids=list(range(NCORES))
    )
    out = np.asarray(res.results[0]["out"]).reshape(B)
    return out.astype(np.asarray(queue_label).dtype)


# revision 11
# speedup vs baseline: 2.7490x; 2.7490x over previous
"""Trainium2 Bass kernel for KL-divergence 1-NN label lookup (AnchorStore).

reference:
    self[k]  = mean_d a[k,d]*log a[k,d]
    cross    = einsum('kd,bd->kb', a, log q) / D
    kl[b,k]  = self[k] - cross[k,b]
    out[b]   = queue_label[argmin_k kl[b,k]]

Strategy (8 NeuronCores, D-sharded):
    Each core owns a D-slice (padded with 1.0 so log()=0 contributes
    nothing). Per core, compute in SUM units (scale-invariant for argmin):
        m_partial[b,k] = sum_d lq[d,b]*at[d,k] - sum_d at[d,k]*log(at[d,k])
    via TensorE: stationary lq tiles [128d,128b] x moving at [128d,512k]
    accumulated in PSUM, plus a "-self" contribution from a (-1)-matrix
    stationary times t = at*log(at). AllReduce(add) the [256,2048] partials,
    then every core computes argmax_k m (== argmin kl) and the label via a
    max/is_equal mask against a broadcast label row. Output int32 labels.
"""

import os
import sys

import numpy as np

sys.path.insert(0, "/opt/trn_rl_repo")

from concourse import bacc, bass, mybir, tile  # noqa: E402
from concourse import bass_utils  # noqa: E402

K = 2048
B = 256
D = 50257
NCORES = 8
DSH = 6400  # padded per-core D-slice (50 tiles of 128)
F32 = mybir.dt.float32


def build(mm_dtype=F32, dsh=DSH, debug_out=False):
    """Build the SPMD Bass graph for one core (all cores identical)."""
    nt = dsh // 128  # d-tiles per core
    kc = K // 512    # k chunks (psum banks per b-tile)
    nc = bacc.Bacc(
        "TRN2", target_bir_lowering=False, debug=False, num_devices=NCORES
    )
    # Matmul-facing tensors carry mm_dtype (float32r requires typed
    # producers so walrus sees rounded operands); all else stays f32.
    at_d = nc.dram_tensor("at", [dsh, K], mm_dtype, kind="ExternalInput")
    qt_d = nc.dram_tensor("qt", [dsh, B], F32, kind="ExternalInput")
    lab_d = nc.dram_tensor("lab1", [128, K], F32, kind="ExternalInput")
    out_d = nc.dram_tensor("out", [B], mybir.dt.int32, kind="ExternalOutput")
    if debug_out:
        mdbg_d = nc.dram_tensor(
            "mdbg", [B // NCORES, K], F32, kind="ExternalOutput"
        )

    LN = mybir.ActivationFunctionType.Ln
    AX = mybir.AxisListType.X
    OP = mybir.AluOpType

    with tile.TileContext(nc) as tc:
        with (
            tc.tile_pool(name="const", bufs=1) as constp,
            tc.tile_pool(name="lqp", bufs=1) as lqp,
            tc.tile_pool(name="qinp", bufs=4) as qinp,
            tc.tile_pool(name="atp", bufs=4) as atp,
            tc.tile_pool(name="latp", bufs=2) as latp,
            tc.tile_pool(name="tpp", bufs=2) as tpp,
            tc.tile_pool(name="msbp", bufs=2) as msbp,
            tc.tile_pool(name="epp", bufs=2) as epp,
            tc.tile_pool(name="psp", bufs=1, space="PSUM") as psp,
            tc.tile_pool(name="dramp", bufs=1, space="DRAM") as dramp,
        ):
            negones = constp.tile([128, 128], mm_dtype)
            if mm_dtype == F32:
                nc.gpsimd.memset(negones[:], -1.0)
            else:
                negones_f = constp.tile([128, 128], F32)
                nc.gpsimd.memset(negones_f[:], -1.0)
                nc.vector.tensor_copy(negones[:], negones_f[:])
            lab1 = constp.tile([128, K], F32)
            nc.sync.dma_start(lab1[:], lab_d[:])

            # lq = log(query^T), resident in SBUF: [128, nt*B]
            lq = lqp.tile([128, nt * B], mm_dtype)
            for t in range(nt):
                qtile = qinp.tile([128, B], F32)
                nc.sync.dma_start(qtile[:], qt_d[t * 128 : (t + 1) * 128, :])
                nc.scalar.activation(lq[:, t * B : (t + 1) * B], qtile[:], LN)

            # PSUM accumulators: [c][bt] -> [128b, 512k] (one bank each)
            pk = [
                [
                    psp.tile(
                        [128, 512],
                        F32,
                        name=f"pk_{c}_{bt}",
                        tag=f"pk_{c}_{bt}",
                    )
                    for bt in range(2)
                ]
                for c in range(kc)
            ]

            for t in range(nt):
                att = atp.tile([128, K], mm_dtype)
                nc.sync.dma_start(att[:], at_d[t * 128 : (t + 1) * 128, :])
                latt = latp.tile([128, K], F32)
                nc.scalar.activation(latt[:], att[:], LN)
                tt = tpp.tile([128, K], mm_dtype)
                nc.vector.tensor_tensor(tt[:], att[:], latt[:], op=OP.mult)
                for bt in range(2):
                    lhs = lq[:, t * B + bt * 128 : t * B + bt * 128 + 128]
                    for c in range(kc):
                        nc.tensor.matmul(
                            pk[c][bt][:],
                            lhs,
                            att[:, c * 512 : (c + 1) * 512],
                            start=(t == 0),
                            stop=False,
                        )
                for bt in range(2):
                    for c in range(kc):
                        nc.tensor.matmul(
                            pk[c][bt][:],
                            negones[:],
                            tt[:, c * 512 : (c + 1) * 512],
                            start=False,
                            stop=(t == nt - 1),
                        )

            # PSUM -> SBUF partials
            m_sb = [
                msbp.tile([128, K], F32, name=f"m_sb{bt}", tag=f"m_sb{bt}")
                for bt in range(2)
            ]
            for bt in range(2):
                for c in range(kc):
                    nc.vector.tensor_copy(
                        m_sb[bt][:, c * 512 : (c + 1) * 512], pk[c][bt][:]
                    )

            # ReduceScatter(add): each core receives the fully-summed m for
            # its 32-query slice (rank r -> queries [32r, 32r+32)).
            BS = B // NCORES
            ar_in = dramp.tile([B, K], F32)
            rs_out = dramp.tile([BS, K], F32)
            for bt in range(2):
                nc.gpsimd.dma_start(
                    ar_in[bt * 128 : (bt + 1) * 128, :], m_sb[bt][:]
                )
            nc.gpsimd.collective_compute(
                "ReduceScatter",
                OP.add,
                replica_groups=[list(range(NCORES))],
                ins=[ar_in.opt()],
                outs=[rs_out.opt()],
            )

            # Local argmax over k + label extraction for this core's 32
            # queries.
            msum = epp.tile([BS, K], F32)
            nc.sync.dma_start(msum[:], rs_out[:])
            if debug_out:
                nc.sync.dma_start(mdbg_d[:], rs_out[:])
            gmax = epp.tile([BS, 1], F32)
            nc.vector.tensor_reduce(gmax[:], msum[:], axis=AX, op=OP.max)
            eq = epp.tile([BS, K], F32)
            nc.vector.tensor_scalar(
                eq[:], msum[:], gmax[:], None, op0=OP.is_equal
            )
            cand = epp.tile([BS, K], F32)
            nc.vector.tensor_tensor(cand[:], eq[:], lab1[:BS, :], op=OP.mult)
            lmax = epp.tile([BS, 1], F32)
            nc.vector.tensor_reduce(lmax[:], cand[:], axis=AX, op=OP.max)
            labf = epp.tile([BS, 1], F32)
            nc.vector.tensor_scalar_add(labf[:], lmax[:], -1.0)

            # AllGather the 32 labels from each core -> [256] on every core.
            ag_in = dramp.tile([BS], F32)
            ag_out = dramp.tile([B], F32)
            nc.sync.dma_start(ag_in[:], labf[:])
            nc.gpsimd.collective_compute(
                "AllGather",
                OP.bypass,
                replica_groups=[list(range(NCORES))],
                ins=[ag_in.opt()],
                outs=[ag_out.opt()],
            )
            labs_f = epp.tile([128, 2], F32)
            nc.sync.dma_start(labs_f[:], ag_out[:])
            labs_i = epp.tile([128, 2], mybir.dt.int32)
            nc.vector.tensor_copy(labs_i[:], labs_f[:])
            nc.sync.dma_start(out_d[:], labs_i[:])

    nc.compile()
    return nc


def shard_inputs(query, queue_anchor, queue_label, dsh=DSH, d_real=D):
    """Host-side layout prep: pad D with 1.0 (log 1 = 0), per-core
    transposed slices, broadcast label row."""
    q = np.asarray(query, np.float32)
    a = np.asarray(queue_anchor, np.float32)
    lab1 = (np.asarray(queue_label).astype(np.float32) + 1.0)[None, :]
    lab1 = np.ascontiguousarray(np.broadcast_to(lab1, (128, lab1.shape[1])))
    in_maps = []
    for c in range(NCORES):
        lo = c * dsh
        hi = min((c + 1) * dsh, d_real)
        at = np.ones((dsh, a.shape[0]), np.float32)
        qt = np.ones((dsh, q.shape[0]), np.float32)
        if hi > lo:
            at[: hi - lo, :] = a[:, lo:hi].T
            qt[: hi - lo, :] = q[:, lo:hi].T
        in_maps.append({"at": at, "qt": qt, "lab1": lab1})
    return in_maps


_NC_CACHE = {}


def _get_nc():
    key = os.environ.get("ANCHOR_MM_DTYPE", "float32r")
    if key not in _NC_CACHE:
        _NC_CACHE[key] = build(mm_dtype=getattr(mybir.dt, key))
    return _NC_CACHE[key]


def kernel(query, queue_anchor, queue_label):
    nc = _get_nc()
    in_maps = shard_inputs(query, queue_anchor, queue_label)
    res = bass_utils.run_bass_kernel_spmd(
        nc, in_maps, core_ids=list(range(NCORES))
    )
    out = np.asarray(res.results[0]["out"]).reshape(B)
    return out.astype(np.asarray(queue_label).dtype)
